# revision 50
# baseline (speedup 1.0000x reference)
"""Trainium2 Bass kernel for the nn_Dynamics problem.

Math (per batch element, d=8, H=128):
  x = X[:, :8], v = X[:, 8:]
  z0 = W0 x + b0; h0 = tanh(z0); z1 = W1 h0 + b1; h1 = tanh(z1)
  a1 = (1-h1^2)*w2;  A0 = W1^T a1;  a0 = (1-h0^2)*A0;  g = W0^T a0
  t0 = W0 v; h0p = (1-h0^2) t0; t1 = W1 h0p; u = h0 (1-h0^2) t0^2
  hvv = sum_h [-2*a1*h1*t1^2 - 2*A0*u]
  force = -(K x + D v)
  out = force - g * (g.force + hvv) / (1 + |g|^2)   (Sherman-Morrison)

Sign convention (saves ops; primed = negated):
  h0p' = (h0^2-1) t0 = -h0p; t1' = -t1; u' = -u; a0' = -a0; g' = -g
  e2' = A0 u' = -e2; hvv = -2 sum(e1) + 2 sum(e2')
  num = hvv - g'.p'; out = p' + num/(1+gg) * g'

Key design points (all measured on HW):
  * X is transposed to [16, B] f16 on the HOST, so the kernel has no
    input transposes; z0/t0/force stream XT slices directly (K=16).
  * a1 is never materialized: A0 = (-W1*w2)^T @ h1^2 + colsum(W1*w2),
    with the constant folded into the Act-engine cast's bias, so the
    A0 stream is h1^2 (one cheap 2x-mode DVE tensor_tensor).
  * Elementwise ops that read f32 PSUM stay fused custom DVE ops (1x);
    e2' = A0f*u' runs on the otherwise-idle Pool engine.
  * The per-element tail (dot products + Sherman-Morrison scale) runs
    batch-major after f16 PE transposes of the packed [p'; g'; hvv]
    block, batched over groups of G=2 tiles and emitted one group late
    so the in-order DVE queue never stalls on it.
  * hvv uses two accumulating PSUM streams (-2 on e1, +2 on e2'), which
    keeps the Pool work off the per-tile critical path.
  * All constants load in 3 packed DMAs split across both HWDGE queues
    (cuts ~10us of serial startup DMA latency).

Sharding: pure data parallel over 8 NeuronCores (8192 rows each), weights
replicated, outputs concatenated.
"""

import os

import ml_dtypes
import numpy as np

import concourse.bacc as bacc
import concourse.bass as bass
import concourse.dve_ops as dve_ops
import concourse.tile as tile
from concourse import mybir
from concourse.bass_utils import run_bass_kernel_spmd
from concourse.dve_ops import DveOp
from concourse.dve_ops import has_src1
from concourse.dve_spec import C0, C1, C2, One, Spec, Src0, Src1, lower, sq
from concourse.dve_uop import DveOpSpec
from concourse.masks import make_identity

F32 = mybir.dt.float32
F16 = mybir.dt.float16
F8 = mybir.dt.float8e4
AX = mybir.AxisListType
OP = mybir.AluOpType
ACT = mybir.ActivationFunctionType

DIM = 8
H = 128
BATCH = 65536
NCORES = 8
BC = BATCH // NCORES          # 8192 rows per core
TW = 512                      # batch tile width
NT = BC // TW                 # 16 tiles per core
NCH = TW // 128               # 4 chunks of 128 per tile
G = 4                         # tiles per tail group
NG = NT // G                  # 4 groups
CPG = G * NCH                 # 16 chunks per group
FMW = 64                      # fm/bm packed rows (force@0, g@32, hvv@64)

LAST_RESULTS = None

# ---------------- custom fused DVE ops ----------------


def _register_op(name, body, reference):
    if name in dve_ops._SUB_OPCODE_FOR_NAME:
        for op in dve_ops.OPS:
            if op.name == name:
                return op
    spec = Spec(body=body, reference=reference)
    shas = {}
    for ver in ("v3", "v4"):
        shas[ver] = DveOpSpec(
            name=name,
            opcode=dve_ops._CUSTOM_DVE_ROW_BASE + len(dve_ops.OPS),
            uops=lower(spec, ver=ver),
            rd1_en=has_src1(spec),
        ).sha(ver)
    op = DveOp(name, spec, subdim=False, uops_sha=shas)
    dve_ops.OPS.append(op)
    dve_ops.CUSTOM_DVE_SPECS[name] = spec
    dve_ops._SUB_OPCODE_FOR_NAME[name] = (
        dve_ops._CUSTOM_DVE_ROW_BASE + len(dve_ops.OPS) - 1
    )
    return op


# h0p' = (h0^2 - 1) * t0     (also a0' = (h0^2 - 1) * A0)
OP_SQM1_MUL = _register_op(
    "ANT_SQM1_MUL",
    (sq(Src0) - One) * Src1,
    lambda in0, in1: (in0 * in0 - 1.0) * in1,
)
# u' = h0 * (h0^2 - 1) * t0^2
OP_UPRIME = _register_op(
    "ANT_UPRIME",
    Src0 * (sq(Src0) - One) * sq(Src1),
    lambda in0, in1: in0 * (in0 * in0 - 1.0) * in1 * in1,
)
# e1 = (1 - h1^2) * w2 * h1 * t1^2
OP_E1F = _register_op(
    "ANT_E1F",
    (One - sq(Src0)) * C0 * Src0 * sq(Src1),
    lambda in0, in1, s0: (1.0 - in0 * in0) * s0 * in0 * in1 * in1,
)
# a1 = (1 - h1^2) * w2
OP_A1F = _register_op(
    "ANT_A1F",
    (One - sq(Src0)) * C0,
    lambda in0, s0: (1.0 - in0 * in0) * s0,
)


def build_nc():
    nc = bacc.Bacc()

    XT = nc.dram_tensor("XT", [2 * DIM, BC], F16, kind="ExternalInput")
    # packed constants: one DMA each for [16,*], [128,*] f16, [128,*] f32
    WA = nc.dram_tensor("WA", [2 * DIM, 2 * H + 32], F16, kind="ExternalInput")
    WB = nc.dram_tensor("WB", [H, 4 * H + 3 * 32], F16, kind="ExternalInput")
    WC = nc.dram_tensor("WC", [H, 4], F32, kind="ExternalInput")
    out = nc.dram_tensor("out", [BC, DIM], F32, kind="ExternalOutput")
    # out natural order: batch b = 512*t + 128*c + p  ->  row (j p), j = 4t+c
    out_r = out.rearrange("(j p) f -> p j f", p=128)

    from contextlib import ExitStack

    with tile.TileContext(nc) as tc, ExitStack() as stk:
        consts = stk.enter_context(tc.tile_pool(name="consts", bufs=1))
        work = stk.enter_context(tc.tile_pool(name="work", bufs=2))
        xtp = stk.enter_context(tc.tile_pool(name="xtp", bufs=3))
        bmp = stk.enter_context(tc.tile_pool(name="bmp", bufs=2, space="PSUM"))
        obp = stk.enter_context(tc.tile_pool(name="obp", bufs=2))
        pzz = stk.enter_context(tc.tile_pool(name="pzz", bufs=2, space="PSUM"))
        ptt = stk.enter_context(tc.tile_pool(name="ptt", bufs=2, space="PSUM"))
        pA0 = stk.enter_context(tc.tile_pool(name="pA0", bufs=1, space="PSUM"))
        pfm = stk.enter_context(tc.tile_pool(name="pfm", bufs=1, space="PSUM"))

        # ---------------- constants (3 packed DMAs) ----------------
        WA_sb = consts.tile([2 * DIM, 2 * H + 32], F16)
        nc.sync.dma_start(out=WA_sb, in_=WA[:, :])
        WB_sb = consts.tile([H, 4 * H + 3 * 32], F16)
        nc.scalar.dma_start(out=WB_sb, in_=WB[:, :])
        WC_sb = consts.tile([H, 4], F32)
        nc.sync.dma_start(out=WC_sb, in_=WC[:, :])
        W0Tx_sb = WA_sb[:, 0:H]
        W0Tv_sb = WA_sb[:, H : 2 * H]
        KDTn_sb = WA_sb[:, 2 * H : 2 * H + 32]
        W1T_sb = WB_sb[:, 0:H]
        W1w_sb = WB_sb[:, H : 2 * H]
        ident_h = WB_sb[:, 2 * H : 3 * H]
        idpad = WB_sb[:, 3 * H : 4 * H]
        W0_sb = idpad[:, 0:32]
        m2_sb = WB_sb[:, 4 * H : 4 * H + 32]
        p2_sb = WB_sb[:, 4 * H + 32 : 4 * H + 64]
        W0b_sb = WB_sb[:, 4 * H + 64 : 4 * H + 96]
        b0_sb = WC_sb[:, 0:1]
        b1_sb = WC_sb[:, 1:2]
        cvec_sb = WC_sb[:, 2:3]
        w2_sb = WC_sb[:, 3:4]

        # ---------------- main loop ----------------
        def emit_tile(t, bm, ti):
            XTs = xtp.tile([2 * DIM, TW], F16, tag="xt", name="XTs")
            nc.sync.dma_start(out=XTs, in_=XT[:, TW * t : TW * (t + 1)])

            z0 = pzz.tile([H, TW], F32, tag="zz", name="z0")
            nc.tensor.matmul(z0, W0Tx_sb, XTs, start=True, stop=True)
            t0 = ptt.tile([H, TW], F32, tag="tt", name="t0")
            nc.tensor.matmul(t0, W0Tv_sb, XTs, start=True, stop=True)

            h0 = work.tile([H, TW], F16, tag="h0", name="h0")
            nc.scalar.activation(h0, z0, ACT.Tanh, bias=b0_sb, scale=1.0)

            # h0p' = (h0^2-1)*t0 ; u' = h0*(h0^2-1)*t0^2
            h0p = work.tile([H, TW], F16, tag="h0p", name="h0p")
            nc.vector._custom_dve(OP_SQM1_MUL, out=h0p, in0=h0, in1=t0[:, :])
            u = work.tile([H, TW], F16, tag="u", name="u")
            nc.vector._custom_dve(OP_UPRIME, out=u, in0=h0, in1=t0[:, :])

            z1 = pzz.tile([H, TW], F32, tag="zz", name="z1")
            nc.tensor.matmul(z1, W1T_sb, h0, start=True, stop=True)
            t1 = ptt.tile([H, TW], F32, tag="tt", name="t1")
            nc.tensor.matmul(t1, W1T_sb, h0p, start=True, stop=True)

            h1 = work.tile([H, TW], F16, tag="h1", name="h1")
            nc.scalar.activation(h1, z1, ACT.Tanh, bias=b1_sb, scale=1.0)

            # e1 = (1-h1^2)*w2*h1*t1^2 ; A0 via folded W1w on h1^2
            h1sq = work.tile([H, TW], F16, tag="h1sq", name="h1sq")
            nc.vector.tensor_mul(h1sq, h1, h1)
            e1 = work.tile([H, TW], F16, tag="e1", name="e1")
            nc.vector._custom_dve(
                OP_E1F, out=e1, in0=h1, in1=t1[:, :], s0=w2_sb[:, 0:1]
            )

            A0 = pA0.tile([H, TW], F32, tag="A0", name="A0")
            nc.tensor.matmul(A0, W1w_sb, h1sq, start=True, stop=True)
            # A0_true = A0_partial + colsum(W1*w2) (Act bias add + cast)
            A0f = work.tile([H, TW], F16, tag="A0f", name="A0f")
            nc.scalar.activation(
                A0f, A0, ACT.Identity, bias=cvec_sb, scale=1.0
            )

            # a0' = (h0^2-1)*A0 ; e2' = A0*u' on Pool ; e12 = e1-e2' on DVE
            a0 = work.tile([H, TW], F16, tag="a0", name="a0")
            nc.vector._custom_dve(OP_SQM1_MUL, out=a0, in0=h0, in1=A0f)
            e2 = work.tile([H, TW], F16, tag="e2", name="e2")
            nc.gpsimd.tensor_mul(e2, A0f, u)

            # packed block: p' rows 0:8, g' rows 32:40, hvv row 64
            # hvv = -2 sum(e1) + 2 sum(e2') via two accumulating streams
            fm = pfm.tile([FMW, TW], F32, tag="fm", name="fm")
            nc.tensor.matmul(
                fm[0:32, :], KDTn_sb, XTs, start=True, stop=False,
                skip_group_check=True,
            )
            nc.tensor.matmul(
                fm[0:32, :], m2_sb, e1, start=False, stop=False,
                skip_group_check=True,
            )
            nc.tensor.matmul(
                fm[32:64, :], W0b_sb, a0, start=True, stop=True,
                tile_position=(0, 32),
            )
            nc.tensor.matmul(
                fm[0:32, :], p2_sb, e2, start=False, stop=True,
                skip_group_check=True,
            )

            E = work.tile([FMW, TW], F16, tag="E", name="E")
            nc.scalar.copy(E, fm[:, :])

            # PE f16 transpose to batch-major, packed into the group tile
            for c in range(NCH):
                j = NCH * ti + c
                nc.tensor.transpose(
                    bm[:, FMW * j : FMW * (j + 1)],
                    E[:, 128 * c : 128 * (c + 1)],
                    ident_h[0:FMW, 0:FMW],
                )

        # ---------------- batched tail over CPG chunks ----------------
        def emit_tail(g, bm):
            def col3(off, w):
                return bass.AP(
                    tensor=bm.tensor,
                    offset=bm.offset + off,
                    ap=[list(bm.ap[0]), [FMW, CPG], [1, w]],
                )

            p3 = col3(0, DIM)
            g3 = col3(32, DIM)
            hv2 = bass.AP(
                tensor=bm.tensor,
                offset=bm.offset + 16,
                ap=[list(bm.ap[0]), [FMW, CPG]],
            )

            # g' columns to SBUF (DVE can't read 2 PSUM inputs per op)
            gS = work.tile([128, CPG * DIM], F16, tag="gS", name="gS")
            gS3 = gS.rearrange("p (c f) -> p c f", f=DIM)
            nc.scalar.copy(gS3, g3)

            gb = work.tile([128, 2 * CPG * DIM], F16, tag="gb", name="gb")
            gb3 = gb.rearrange("p (q c f) -> p (q c) f", f=DIM, q=2)
            nc.gpsimd.tensor_mul(gb3[:, 0:CPG, :], gS3, gS3)
            nc.vector.tensor_mul(gb3[:, CPG : 2 * CPG, :], gS3, p3)
            red = work.tile([128, 2 * CPG], F32, tag="red", name="red")
            nc.vector.tensor_reduce(red, gb3, axis=AX.X, op=OP.add)
            den = work.tile([128, CPG], F32, tag="den", name="den")
            nc.vector.tensor_scalar_add(den, red[:, 0:CPG], 1.0)
            gps = red[:, CPG : 2 * CPG]
            num = work.tile([128, CPG], F32, tag="num", name="num")
            nc.vector.tensor_sub(num, hv2, gps)
            rec = work.tile([128, CPG], F32, tag="rec", name="rec")
            nc.vector.reciprocal(rec, den)
            s4 = work.tile([128, CPG], F32, tag="s4", name="s4")
            nc.vector.tensor_mul(s4, num, rec)
            s4b = bass.AP(
                tensor=s4.tensor,
                offset=s4.offset,
                ap=[list(s4.ap[0]), [1, CPG], [0, DIM]],
            )
            su = work.tile([128, CPG * DIM], F32, tag="su", name="su")
            su3 = su.rearrange("p (c f) -> p c f", f=DIM)
            nc.gpsimd.tensor_mul(su3, gS3, s4b)
            ob = obp.tile([128, CPG * DIM], F32, tag="ob", name="ob")
            nc.vector.tensor_add(
                ob.rearrange("p (c f) -> p c f", f=DIM), p3, su3
            )
            nc.sync.dma_start(
                out=out_r[:, CPG * g : CPG * (g + 1), :], in_=ob
            )

        # emit tails one group late so the in-order DVE queue never
        # stalls waiting for a group's transposes
        bm_tiles = {}
        for t in range(NT):
            g = t // G
            if t % G == 0:
                bm_tiles[g] = bmp.tile(
                    [128, CPG * FMW], F16, tag="bm", name="bm"
                )
            emit_tile(t, bm_tiles[g], t % G)
            if t % G == G - 1 and g >= 1:
                emit_tail(g - 1, bm_tiles.pop(g - 1))
        emit_tail(NG - 1, bm_tiles.pop(NG - 1))

    if not nc.is_finalized():
        nc.finalize()

    return nc


_NC_CACHE = None


def _install_ntff_shim():
    """Register the axon NTFF profile hook (missing antenv.axon_hooks shim)."""
    import sys
    import types

    if "antenv.axon_hooks" in sys.modules:
        return
    try:
        sys.path.insert(0, "/root/.axon_site")
        from trn_agent_boot.trn_boot import _ntff_profile_via_ctypes

        hook = _ntff_profile_via_ctypes("/opt/axon/libaxon_pjrt.so")
        mod = types.ModuleType("antenv.axon_hooks")
        mod.get_axon_ntff_profile_hook = lambda: hook
        sys.modules["antenv.axon_hooks"] = mod
    except Exception:
        pass


def kernel(**inputs):
    global LAST_RESULTS, _NC_CACHE
    trace = bool(int(os.environ.get("KERNEL_TRACE", "0")))
    if trace:
        _install_ntff_shim()
    if _NC_CACHE is None:
        _NC_CACHE = build_nc()
    nc = _NC_CACHE

    X = np.ascontiguousarray(inputs["X"], dtype=np.float32)
    K = np.asarray(inputs["K"], np.float32)
    D = np.asarray(inputs["D"], np.float32)
    W0 = np.asarray(inputs["W0"], np.float32)
    W1 = np.asarray(inputs["W1"], np.float32)
    W2 = np.asarray(inputs["W2"], np.float32)
    w0pad = np.zeros((H, 32), np.float32)
    w0pad[:, 0:DIM] = W0
    w0tx = np.zeros((2 * DIM, H), np.float32)
    w0tx[0:DIM] = W0.T
    w0tv = np.zeros((2 * DIM, H), np.float32)
    w0tv[DIM:] = W0.T
    m2 = np.zeros((H, 32), np.float32)
    m2[:, 16] = -2.0
    p2 = np.zeros((H, 32), np.float32)
    p2[:, 16] = 2.0
    kdt = np.zeros((2 * DIM, 32), np.float32)
    kdt[:, 0:DIM] = np.concatenate([-K.T, -D.T], axis=0)
    w2v = W2.reshape(H)
    w1w = -(W1 * w2v[:, None])
    cv = (W1 * w2v[:, None]).sum(axis=0).astype(np.float32)
    wa = np.concatenate([w0tx, w0tv, kdt], axis=1).astype(np.float16)
    wb = np.concatenate(
        [
            W1.T,
            w1w,
            np.eye(H, dtype=np.float32),
            np.zeros((H, H), np.float32),
            m2,
            p2,
            w0pad,
        ],
        axis=1,
    ).astype(np.float16)
    wc = np.stack(
        [
            np.asarray(inputs["b0"], np.float32),
            np.asarray(inputs["b1"], np.float32),
            cv,
            W2.reshape(H).astype(np.float32),
        ],
        axis=1,
    ).astype(np.float32)
    shared = {
        "WA": np.ascontiguousarray(wa),
        "WB": np.ascontiguousarray(wb),
        "WC": np.ascontiguousarray(wc),
    }
    in_maps = []
    for i in range(NCORES):
        xt = np.ascontiguousarray(X[i * BC : (i + 1) * BC].T).astype(np.float16)
        m = {"XT": xt}
        m.update(shared)
        in_maps.append(m)

    res = run_bass_kernel_spmd(
        nc, in_maps, core_ids=list(range(NCORES)), trace=trace
    )
    LAST_RESULTS = res
    out_full = np.concatenate(
        [res.results[i]["out"] for i in range(NCORES)], axis=0
    )
    return out_full.astype(np.float32)


# revision 51
# speedup vs baseline: 1.0484x; 1.0484x over previous
"""Trainium2 Bass kernel for the nn_Dynamics problem.

Math (per batch element, d=8, H=128):
  x = X[:, :8], v = X[:, 8:]
  z0 = W0 x + b0; h0 = tanh(z0); z1 = W1 h0 + b1; h1 = tanh(z1)
  a1 = (1-h1^2)*w2;  A0 = W1^T a1;  a0 = (1-h0^2)*A0;  g = W0^T a0
  t0 = W0 v; h0p = (1-h0^2) t0; t1 = W1 h0p; u = h0 (1-h0^2) t0^2
  hvv = sum_h [-2*a1*h1*t1^2 - 2*A0*u]
  force = -(K x + D v)
  out = force - g * (g.force + hvv) / (1 + |g|^2)   (Sherman-Morrison)

Sign convention (saves ops; primed = negated):
  h0p' = (h0^2-1) t0 = -h0p; t1' = -t1; u' = -u; a0' = -a0; g' = -g
  e2' = A0 u' = -e2; hvv = -2 sum(e1) + 2 sum(e2')
  num = hvv - g'.p'; out = p' + num/(1+gg) * g'

Key design points (all measured on HW):
  * X is transposed to [16, B] f16 on the HOST, so the kernel has no
    input transposes; z0/t0/force stream XT slices directly (K=16).
  * a1 is never materialized: A0 = (-W1*w2)^T @ h1^2 + colsum(W1*w2),
    with the constant folded into the Act-engine cast's bias, so the
    A0 stream is h1^2 (one cheap 2x-mode DVE tensor_tensor).
  * Elementwise ops that read f32 PSUM stay fused custom DVE ops (1x);
    e2' = A0f*u' runs on the otherwise-idle Pool engine.
  * The per-element tail (dot products + Sherman-Morrison scale) runs
    batch-major after f16 PE transposes of the packed [p'; g'; hvv]
    block, batched over groups of G=2 tiles and emitted one group late
    so the in-order DVE queue never stalls on it.
  * hvv uses two accumulating PSUM streams (-2 on e1, +2 on e2'), which
    keeps the Pool work off the per-tile critical path.
  * All constants load in 3 packed DMAs split across both HWDGE queues
    (cuts ~10us of serial startup DMA latency).

Sharding: pure data parallel over 8 NeuronCores (8192 rows each), weights
replicated, outputs concatenated.
"""

import os

import ml_dtypes
import numpy as np

import concourse.bacc as bacc
import concourse.bass as bass
import concourse.dve_ops as dve_ops
import concourse.tile as tile
from concourse import mybir
from concourse.bass_utils import run_bass_kernel_spmd
from concourse.dve_ops import DveOp
from concourse.dve_ops import has_src1
from concourse.dve_spec import C0, C1, C2, One, Spec, Src0, Src1, lower, sq
from concourse.dve_uop import DveOpSpec
from concourse.masks import make_identity

F32 = mybir.dt.float32
F16 = mybir.dt.float16
F8 = mybir.dt.float8e4
AX = mybir.AxisListType
OP = mybir.AluOpType
ACT = mybir.ActivationFunctionType

DIM = 8
H = 128
BATCH = 65536
NCORES = 8
BC = BATCH // NCORES          # 8192 rows per core
TW = 512                      # batch tile width
NT = BC // TW                 # 16 tiles per core
NCH = TW // 128               # 4 chunks of 128 per tile
G = 2                         # tiles per tail group
NG = NT // G                  # 8 groups
CPG = G * NCH                 # 8 chunks per group
FMW = 96                      # fm/bm packed rows (force@0, g@32, hvv@64)

LAST_RESULTS = None

# ---------------- custom fused DVE ops ----------------


def _register_op(name, body, reference):
    if name in dve_ops._SUB_OPCODE_FOR_NAME:
        for op in dve_ops.OPS:
            if op.name == name:
                return op
    spec = Spec(body=body, reference=reference)
    shas = {}
    for ver in ("v3", "v4"):
        shas[ver] = DveOpSpec(
            name=name,
            opcode=dve_ops._CUSTOM_DVE_ROW_BASE + len(dve_ops.OPS),
            uops=lower(spec, ver=ver),
            rd1_en=has_src1(spec),
        ).sha(ver)
    op = DveOp(name, spec, subdim=False, uops_sha=shas)
    dve_ops.OPS.append(op)
    dve_ops.CUSTOM_DVE_SPECS[name] = spec
    dve_ops._SUB_OPCODE_FOR_NAME[name] = (
        dve_ops._CUSTOM_DVE_ROW_BASE + len(dve_ops.OPS) - 1
    )
    return op


# h0p' = (h0^2 - 1) * t0     (also a0' = (h0^2 - 1) * A0)
OP_SQM1_MUL = _register_op(
    "ANT_SQM1_MUL",
    (sq(Src0) - One) * Src1,
    lambda in0, in1: (in0 * in0 - 1.0) * in1,
)
# u' = h0 * (h0^2 - 1) * t0^2
OP_UPRIME = _register_op(
    "ANT_UPRIME",
    Src0 * (sq(Src0) - One) * sq(Src1),
    lambda in0, in1: in0 * (in0 * in0 - 1.0) * in1 * in1,
)
# e1 = (1 - h1^2) * w2 * h1 * t1^2
OP_E1F = _register_op(
    "ANT_E1F",
    (One - sq(Src0)) * C0 * Src0 * sq(Src1),
    lambda in0, in1, s0: (1.0 - in0 * in0) * s0 * in0 * in1 * in1,
)
# a1 = (1 - h1^2) * w2
OP_A1F = _register_op(
    "ANT_A1F",
    (One - sq(Src0)) * C0,
    lambda in0, s0: (1.0 - in0 * in0) * s0,
)


def build_nc():
    nc = bacc.Bacc()

    XT = nc.dram_tensor("XT", [2 * DIM, BC], F16, kind="ExternalInput")
    # packed constants: one DMA each for [16,*], [128,*] f16, [128,*] f32
    WA = nc.dram_tensor("WA", [2 * DIM, 2 * H + DIM], F16, kind="ExternalInput")
    WB = nc.dram_tensor("WB", [H, 4 * H + 3 * 32], F16, kind="ExternalInput")
    WC = nc.dram_tensor("WC", [H, 4], F32, kind="ExternalInput")
    out = nc.dram_tensor("out", [BC, DIM], F32, kind="ExternalOutput")
    # out natural order: batch b = 512*t + 128*c + p  ->  row (j p), j = 4t+c
    out_r = out.rearrange("(j p) f -> p j f", p=128)

    from contextlib import ExitStack

    with tile.TileContext(nc) as tc, ExitStack() as stk:
        consts = stk.enter_context(tc.tile_pool(name="consts", bufs=1))
        work = stk.enter_context(tc.tile_pool(name="work", bufs=2))
        xtp = stk.enter_context(tc.tile_pool(name="xtp", bufs=3))
        bmp = stk.enter_context(tc.tile_pool(name="bmp", bufs=2, space="PSUM"))
        obp = stk.enter_context(tc.tile_pool(name="obp", bufs=2))
        pzz = stk.enter_context(tc.tile_pool(name="pzz", bufs=2, space="PSUM"))
        ptt = stk.enter_context(tc.tile_pool(name="ptt", bufs=2, space="PSUM"))
        pA0 = stk.enter_context(tc.tile_pool(name="pA0", bufs=1, space="PSUM"))
        pfm = stk.enter_context(tc.tile_pool(name="pfm", bufs=1, space="PSUM"))

        # ---------------- constants (3 packed DMAs) ----------------
        WA_sb = consts.tile([2 * DIM, 2 * H + DIM], F16)
        nc.sync.dma_start(out=WA_sb, in_=WA[:, :])
        WB_sb = consts.tile([H, 4 * H + 3 * 32], F16)
        nc.scalar.dma_start(out=WB_sb, in_=WB[:, :])
        WC_sb = consts.tile([H, 4], F32)
        nc.sync.dma_start(out=WC_sb, in_=WC[:, :])
        W0Tx_sb = WA_sb[:, 0:H]
        W0Tv_sb = WA_sb[:, H : 2 * H]
        KDTn_sb = WA_sb[:, 2 * H : 2 * H + DIM]
        W1T_sb = WB_sb[:, 0:H]
        W1w_sb = WB_sb[:, H : 2 * H]
        ident_h = WB_sb[:, 2 * H : 3 * H]
        idpad = WB_sb[:, 3 * H : 4 * H]
        W0_sb = idpad[:, 0:32]
        m2_sb = WB_sb[:, 4 * H : 4 * H + 32]
        p2_sb = WB_sb[:, 4 * H + 32 : 4 * H + 64]
        W0b_sb = WB_sb[:, 4 * H + 64 : 4 * H + 96]
        b0_sb = WC_sb[:, 0:1]
        b1_sb = WC_sb[:, 1:2]
        cvec_sb = WC_sb[:, 2:3]
        w2_sb = WC_sb[:, 3:4]

        # ---------------- main loop ----------------
        def emit_tile(t, bm, ti):
            XTs = xtp.tile([2 * DIM, TW], F16, tag="xt", name="XTs")
            nc.sync.dma_start(out=XTs, in_=XT[:, TW * t : TW * (t + 1)])

            z0 = pzz.tile([H, TW], F32, tag="zz", name="z0")
            nc.tensor.matmul(z0, W0Tx_sb, XTs, start=True, stop=True)
            t0 = ptt.tile([H, TW], F32, tag="tt", name="t0")
            nc.tensor.matmul(t0, W0Tv_sb, XTs, start=True, stop=True)

            h0 = work.tile([H, TW], F16, tag="h0", name="h0")
            nc.scalar.activation(h0, z0, ACT.Tanh, bias=b0_sb, scale=1.0)

            # h0p' = (h0^2-1)*t0 ; u' = h0*(h0^2-1)*t0^2
            h0p = work.tile([H, TW], F16, tag="h0p", name="h0p")
            nc.vector._custom_dve(OP_SQM1_MUL, out=h0p, in0=h0, in1=t0[:, :])
            u = work.tile([H, TW], F16, tag="u", name="u")
            nc.vector._custom_dve(OP_UPRIME, out=u, in0=h0, in1=t0[:, :])

            z1 = pzz.tile([H, TW], F32, tag="zz", name="z1")
            nc.tensor.matmul(z1, W1T_sb, h0, start=True, stop=True)
            t1 = ptt.tile([H, TW], F32, tag="tt", name="t1")
            nc.tensor.matmul(t1, W1T_sb, h0p, start=True, stop=True)

            h1 = work.tile([H, TW], F16, tag="h1", name="h1")
            nc.scalar.activation(h1, z1, ACT.Tanh, bias=b1_sb, scale=1.0)

            # e1 = (1-h1^2)*w2*h1*t1^2 ; A0 via folded W1w on h1^2
            h1sq = work.tile([H, TW], F16, tag="h1sq", name="h1sq")
            nc.vector.tensor_mul(h1sq, h1, h1)
            e1 = work.tile([H, TW], F16, tag="e1", name="e1")
            nc.vector._custom_dve(
                OP_E1F, out=e1, in0=h1, in1=t1[:, :], s0=w2_sb[:, 0:1]
            )

            A0 = pA0.tile([H, TW], F32, tag="A0", name="A0")
            nc.tensor.matmul(A0, W1w_sb, h1sq, start=True, stop=True)
            # A0_true = A0_partial + colsum(W1*w2) (Act bias add + cast)
            A0f = work.tile([H, TW], F16, tag="A0f", name="A0f")
            nc.scalar.activation(
                A0f, A0, ACT.Identity, bias=cvec_sb, scale=1.0
            )

            # a0' = (h0^2-1)*A0 ; e2' = A0*u' on Pool ; e12 = e1-e2' on DVE
            a0 = work.tile([H, TW], F16, tag="a0", name="a0")
            nc.vector._custom_dve(OP_SQM1_MUL, out=a0, in0=h0, in1=A0f)
            e2 = work.tile([H, TW], F16, tag="e2", name="e2")
            nc.gpsimd.tensor_mul(e2, A0f, u)

            # packed block: p' rows 0:8, g' rows 32:40, hvv row 64
            # hvv = -2 sum(e1) + 2 sum(e2') via two accumulating streams
            fm = pfm.tile([FMW, TW], F32, tag="fm", name="fm")
            nc.tensor.matmul(
                fm[0:DIM, :], KDTn_sb, XTs, start=True, stop=True
            )
            nc.tensor.matmul(
                fm[32:64, :], W0b_sb, a0, start=True, stop=True,
                tile_position=(0, 32),
            )
            nc.tensor.matmul(
                fm[64:96, :], m2_sb, e1, start=True, stop=False,
                tile_position=(0, 64),
            )
            nc.tensor.matmul(
                fm[64:96, :], p2_sb, e2, start=False, stop=True,
                tile_position=(0, 64),
            )

            E = work.tile([FMW, TW], F16, tag="E", name="E")
            nc.scalar.copy(E, fm[:, :])

            # PE f16 transpose to batch-major, packed into the group tile
            for c in range(NCH):
                j = NCH * ti + c
                nc.tensor.transpose(
                    bm[:, FMW * j : FMW * (j + 1)],
                    E[:, 128 * c : 128 * (c + 1)],
                    ident_h[0:FMW, 0:FMW],
                )

        # ---------------- batched tail over CPG chunks ----------------
        def emit_tail(g, bm):
            def col3(off, w):
                return bass.AP(
                    tensor=bm.tensor,
                    offset=bm.offset + off,
                    ap=[list(bm.ap[0]), [FMW, CPG], [1, w]],
                )

            p3 = col3(0, DIM)
            g3 = col3(32, DIM)
            hv2 = bass.AP(
                tensor=bm.tensor,
                offset=bm.offset + 64,
                ap=[list(bm.ap[0]), [FMW, CPG]],
            )

            # g' columns to SBUF (DVE can't read 2 PSUM inputs per op)
            gS = work.tile([128, CPG * DIM], F16, tag="gS", name="gS")
            gS3 = gS.rearrange("p (c f) -> p c f", f=DIM)
            nc.scalar.copy(gS3, g3)

            gb = work.tile([128, 2 * CPG * DIM], F16, tag="gb", name="gb")
            gb3 = gb.rearrange("p (q c f) -> p (q c) f", f=DIM, q=2)
            nc.gpsimd.tensor_mul(gb3[:, 0:CPG, :], gS3, gS3)
            nc.vector.tensor_mul(gb3[:, CPG : 2 * CPG, :], gS3, p3)
            red = work.tile([128, 2 * CPG], F32, tag="red", name="red")
            nc.vector.tensor_reduce(red, gb3, axis=AX.X, op=OP.add)
            den = work.tile([128, CPG], F32, tag="den", name="den")
            nc.vector.tensor_scalar_add(den, red[:, 0:CPG], 1.0)
            gps = red[:, CPG : 2 * CPG]
            num = work.tile([128, CPG], F32, tag="num", name="num")
            nc.vector.tensor_sub(num, hv2, gps)
            rec = work.tile([128, CPG], F32, tag="rec", name="rec")
            nc.vector.reciprocal(rec, den)
            s4 = work.tile([128, CPG], F32, tag="s4", name="s4")
            nc.vector.tensor_mul(s4, num, rec)
            s4b = bass.AP(
                tensor=s4.tensor,
                offset=s4.offset,
                ap=[list(s4.ap[0]), [1, CPG], [0, DIM]],
            )
            su = work.tile([128, CPG * DIM], F32, tag="su", name="su")
            su3 = su.rearrange("p (c f) -> p c f", f=DIM)
            nc.gpsimd.tensor_mul(su3, gS3, s4b)
            ob = obp.tile([128, CPG * DIM], F32, tag="ob", name="ob")
            nc.vector.tensor_add(
                ob.rearrange("p (c f) -> p c f", f=DIM), p3, su3
            )
            nc.sync.dma_start(
                out=out_r[:, CPG * g : CPG * (g + 1), :], in_=ob
            )

        # emit tails one group late so the in-order DVE queue never
        # stalls waiting for a group's transposes
        bm_tiles = {}
        for t in range(NT):
            g = t // G
            if t % G == 0:
                bm_tiles[g] = bmp.tile(
                    [128, CPG * FMW], F16, tag="bm", name="bm"
                )
            emit_tile(t, bm_tiles[g], t % G)
            if t % G == G - 1 and g >= 1:
                emit_tail(g - 1, bm_tiles.pop(g - 1))
        emit_tail(NG - 1, bm_tiles.pop(NG - 1))

    if not nc.is_finalized():
        nc.finalize()

    return nc


_NC_CACHE = None


def _install_ntff_shim():
    """Register the axon NTFF profile hook (missing antenv.axon_hooks shim)."""
    import sys
    import types

    if "antenv.axon_hooks" in sys.modules:
        return
    try:
        sys.path.insert(0, "/root/.axon_site")
        from trn_agent_boot.trn_boot import _ntff_profile_via_ctypes

        hook = _ntff_profile_via_ctypes("/opt/axon/libaxon_pjrt.so")
        mod = types.ModuleType("antenv.axon_hooks")
        mod.get_axon_ntff_profile_hook = lambda: hook
        sys.modules["antenv.axon_hooks"] = mod
    except Exception:
        pass


def kernel(**inputs):
    global LAST_RESULTS, _NC_CACHE
    trace = bool(int(os.environ.get("KERNEL_TRACE", "0")))
    if trace:
        _install_ntff_shim()
    if _NC_CACHE is None:
        _NC_CACHE = build_nc()
    nc = _NC_CACHE

    X = np.ascontiguousarray(inputs["X"], dtype=np.float32)
    K = np.asarray(inputs["K"], np.float32)
    D = np.asarray(inputs["D"], np.float32)
    W0 = np.asarray(inputs["W0"], np.float32)
    W1 = np.asarray(inputs["W1"], np.float32)
    W2 = np.asarray(inputs["W2"], np.float32)
    w0pad = np.zeros((H, 32), np.float32)
    w0pad[:, 0:DIM] = W0
    w0tx = np.zeros((2 * DIM, H), np.float32)
    w0tx[0:DIM] = W0.T
    w0tv = np.zeros((2 * DIM, H), np.float32)
    w0tv[DIM:] = W0.T
    m2 = np.zeros((H, 32), np.float32)
    m2[:, 0] = -2.0
    p2 = np.zeros((H, 32), np.float32)
    p2[:, 0] = 2.0
    kdt = np.concatenate([-K.T, -D.T], axis=0)
    w2v = W2.reshape(H)
    w1w = -(W1 * w2v[:, None])
    cv = (W1 * w2v[:, None]).sum(axis=0).astype(np.float32)
    wa = np.concatenate([w0tx, w0tv, kdt], axis=1).astype(np.float16)
    wb = np.concatenate(
        [
            W1.T,
            w1w,
            np.eye(H, dtype=np.float32),
            np.zeros((H, H), np.float32),
            m2,
            p2,
            w0pad,
        ],
        axis=1,
    ).astype(np.float16)
    wc = np.stack(
        [
            np.asarray(inputs["b0"], np.float32),
            np.asarray(inputs["b1"], np.float32),
            cv,
            W2.reshape(H).astype(np.float32),
        ],
        axis=1,
    ).astype(np.float32)
    shared = {
        "WA": np.ascontiguousarray(wa),
        "WB": np.ascontiguousarray(wb),
        "WC": np.ascontiguousarray(wc),
    }
    in_maps = []
    for i in range(NCORES):
        xt = np.ascontiguousarray(X[i * BC : (i + 1) * BC].T).astype(np.float16)
        m = {"XT": xt}
        m.update(shared)
        in_maps.append(m)

    res = run_bass_kernel_spmd(
        nc, in_maps, core_ids=list(range(NCORES)), trace=trace
    )
    LAST_RESULTS = res
    out_full = np.concatenate(
        [res.results[i]["out"] for i in range(NCORES)], axis=0
    )
    return out_full.astype(np.float32)
